# revision 21
# baseline (speedup 1.0000x reference)
"""Trainium2 Bass kernel for CentroidsFlowAD (retrieval_knn, K=1).

Math: for each embedding row e (B*N rows of dim D=1024) and centroid bank
C [M=2048, D], the reference computes min_m sqrt(max(||e||^2 + ||c_m||^2
- 2 e.c_m, 0)). With K_NEIGHBORS=1 the softmin weighting is exactly 1, so
the output is just the distance to the nearest centroid, reshaped to
[B, 1, 56, 56].

Strategy (data-parallel over batch across 8 cores, centroids replicated):
  - host: split embeds by batch (4 samples -> 12544 rows per core), cast
    to fp8 e4m3 (TRN FP8_EXP4) and transpose to [D, R].
  - bias fold ("dim_fold"): contraction dim 1023 is sacrificed to carry
    the per-centroid bias: e''[1023]=1, c''[1023]=delta_m where
    delta_m = 512 - ||c_m||^2/2; host feat = ||e||^2 + 1024 compensates
    the 512 centering. The kernel then just computes
    hmax = max_m(e''.c''_m) and dist = sqrt(feat - 2*hmax): no
    per-element bias stage on any engine. Dropping dim 1023 from the
    cross term costs ~5e-3 max rel err (tolerance 2e-2).
  - device: fp8 DoubleRow matmuls (2 fp8/cell/cycle, [128,2,*] AP
    pairs), 4 k-pair chunks x 2 n-chunks of 512 into 4 rotating PSUM
    buffers of [128,1024] (deep PE->DVE pipelining); DVE max-reduces
    each buffer straight from PSUM (the 1 elem/cycle/partition PSUM
    read is the measured bottleneck at ~209us/core; PE hides under it).
  - host: gather per-core [128, NT] outputs, unpermute, reshape.

Measured on the staged trn2 cores: bf16 baseline 970us -> fp8 DoubleRow
+ 2-op DVE 437us -> ACT-bias-preload 274us -> this (no 3-engine chain)
~211us. DVE PSUM-read floor is ~209us/core.
"""

import numpy as np
import ml_dtypes

import concourse.bass as bass
import concourse.mybir as mybir
import concourse.tile as tile
from concourse import bacc
from concourse.bass_utils import run_bass_kernel_spmd

# Problem constants (hardcoded per harness contract)
B, N, D, M = 32, 3136, 1024, 2048
N_CORES = 8
B_PER_CORE = B // N_CORES            # 4
R = B_PER_CORE * N                   # 12544 rows per core
NT = R // 128                        # 98 row tiles per core
KC = D // 128                        # 8 contraction chunks of 128
KP = KC // 2                         # 4 DoubleRow k-pair chunks of 256
FP_H = 56
BIAS_CENTER = 512.0                  # delta = BIAS_CENTER - csq/2

FP8 = mybir.dt.float8e4
F32 = mybir.dt.float32
NP_FP8 = ml_dtypes.float8_e4m3       # == mybir.dt.np(float8e4); TRN E4M3


def build_program(n_row_tiles=NT, block_tiles=14, n_iters=1, n_devices=N_CORES,
                  enable_asserts=False, warmup=False, newton=True,
                  act_split=False):
    """Build + compile the SPMD bass program.

    n_row_tiles: row tiles (128 rows each) processed per core.
    block_tiles: row tiles per DMA block (must divide n_row_tiles).
    n_iters: repeat whole compute (for loop-delta timing), python-unrolled.
    """
    assert n_row_tiles % block_tiles == 0
    n_blocks = n_row_tiles // block_tiles
    rows = n_row_tiles * 128
    blk = block_tiles * 128
    DR = mybir.MatmulPerfMode.DoubleRow

    nc = bacc.Bacc("TRN2", target_bir_lowering=False, debug=False,
                   num_devices=n_devices, enable_asserts=enable_asserts)

    et = nc.dram_tensor("et", [D, rows], FP8, kind="ExternalInput").ap()
    ct = nc.dram_tensor("ct", [D, M], FP8, kind="ExternalInput").ap()
    feat = nc.dram_tensor("feat", [128, n_row_tiles], F32,
                          kind="ExternalInput").ap()
    out = nc.dram_tensor("out", [128, n_row_tiles], F32,
                         kind="ExternalOutput").ap()

    with tile.TileContext(nc) as tc:
        with (
            tc.tile_pool(name="const", bufs=1) as const_pool,
            tc.tile_pool(name="etp", bufs=2) as et_pool,
            tc.tile_pool(name="psum", bufs=4, space="PSUM") as psum_pool,
            tc.tile_pool(name="hb", bufs=2) as hb_pool,
            tc.tile_pool(name="epi", bufs=1) as epi_pool,
        ):
            ct_sb = const_pool.tile([128, KC, M], FP8)
            feat_sb = const_pool.tile([128, n_row_tiles], F32)
            hmax_sb = const_pool.tile([128, n_row_tiles], F32)
            hm2_sb = const_pool.tile([128, 2, n_row_tiles], F32)
            for k in range(KC):
                nc.sync.dma_start(ct_sb[:, k, :], ct[k * 128:(k + 1) * 128, :])
            nc.sync.dma_start(feat_sb[:], feat[:, :])

            if warmup:
                # zero-weight matmuls through all 4 psum buffers before the
                # main loop (mirrors the structure that measured fastest)
                zw_sb = const_pool.tile([1, 2, 128], FP8)
                nc.vector.memset(zw_sb[:], 0.0)
                for _ in range(4):
                    ps = psum_pool.tile([128, 1024], F32)
                    for n2 in range(2):
                        nc.tensor.matmul(
                            ps[:, n2 * 512:(n2 + 1) * 512], zw_sb[:],
                            ct_sb[0:1, 0:2, n2 * 512:(n2 + 1) * 512],
                            start=True, stop=True,
                            perf_mode=mybir.MatmulPerfMode.DoubleRow)
                    nc.vector.tensor_reduce(
                        hmax_sb[:, 0:1], ps[:, 0:4],
                        mybir.AxisListType.X, mybir.AluOpType.max)

            def body(_it=None):
                for b in range(n_blocks):
                    et_sb = et_pool.tile([128, KC, blk], FP8)
                    for k in range(KC):
                        nc.sync.dma_start(
                            et_sb[:, k, :],
                            et[k * 128:(k + 1) * 128, b * blk:(b + 1) * blk])
                    for j in range(block_tiles):
                        t = b * block_tiles + j
                        for h in range(2):
                            ps = psum_pool.tile([128, 1024], F32)
                            for kp in range(KP):
                                lhsT = et_sb[:, 2 * kp:2 * kp + 2,
                                             j * 128:(j + 1) * 128]
                                for n2 in range(2):
                                    n = 2 * h + n2
                                    # skip_group_check is load-bearing for
                                    # performance here: without it the psum
                                    # accumulation-group bookkeeping emits a
                                    # schedule that nearly doubles wall time
                                    # (406us vs 211us measured)
                                    nc.tensor.matmul(
                                        ps[:, n2 * 512:(n2 + 1) * 512], lhsT,
                                        ct_sb[:, 2 * kp:2 * kp + 2,
                                              n * 512:(n + 1) * 512],
                                        start=(kp == 0), stop=(kp == KP - 1),
                                        perf_mode=DR, skip_group_check=True)
                            if act_split and h == 1:
                                # ACT (otherwise idle) downcast-copies this
                                # half to SBUF bf16; DVE then reduces it in
                                # its 2x packed mode instead of the
                                # 1 elem/cycle PSUM port
                                hb = hb_pool.tile([128, 1024],
                                                  mybir.dt.bfloat16)
                                nc.scalar.activation(
                                    hb[:], ps[:],
                                    mybir.ActivationFunctionType.Copy)
                                nc.vector.tensor_reduce(
                                    hm2_sb[:, h, t:t + 1], hb[:],
                                    mybir.AxisListType.X, mybir.AluOpType.max)
                            else:
                                nc.vector.tensor_reduce(
                                    hm2_sb[:, h, t:t + 1], ps[:],
                                    mybir.AxisListType.X, mybir.AluOpType.max)

                # epilogue: dist = sqrt(max(feat - 2*hmax, eps)), Newton-refined
                nc.vector.tensor_tensor(
                    hmax_sb[:], hm2_sb[:, 0, :], hm2_sb[:, 1, :],
                    mybir.AluOpType.max)
                d2 = epi_pool.tile([128, n_row_tiles], F32)
                nc.vector.scalar_tensor_tensor(
                    out=d2[:], in0=hmax_sb[:], scalar=-2.0, in1=feat_sb[:],
                    op0=mybir.AluOpType.mult, op1=mybir.AluOpType.add)
                d2c = epi_pool.tile([128, n_row_tiles], F32)
                nc.vector.tensor_scalar_max(d2c[:], d2[:], 1.0e-12)
                s0 = epi_pool.tile([128, n_row_tiles], F32)
                nc.scalar.activation(s0[:], d2c[:],
                                     mybir.ActivationFunctionType.Sqrt)
                if newton:
                    # one Newton step to refine the ACT LUT sqrt (~1e-4 ->
                    # ~1e-7); unneeded at the 2e-2 gate, costs serial DVE ops
                    rcp = epi_pool.tile([128, n_row_tiles], F32)
                    nc.vector.reciprocal(rcp[:], s0[:])
                    q = epi_pool.tile([128, n_row_tiles], F32)
                    nc.vector.tensor_mul(q[:], d2c[:], rcp[:])
                    sq = epi_pool.tile([128, n_row_tiles], F32)
                    nc.vector.tensor_add(sq[:], s0[:], q[:])
                    res = epi_pool.tile([128, n_row_tiles], F32)
                    nc.vector.tensor_scalar_mul(res[:], sq[:], 0.5)
                else:
                    res = s0
                nc.sync.dma_start(out[:, :], res[:])

            # python-unrolled repetitions (For_i's back-edge machinery has
            # crashed the exec unit on this terminal; unrolled is safe)
            for _ in range(n_iters):
                body()

    nc.compile()
    return nc


_NC_CACHE = {}


def _get_program(key=(NT, 14, 1, N_CORES)):
    if key not in _NC_CACHE:
        _NC_CACHE[key] = build_program(*key)
    return _NC_CACHE[key]


def prep_centroids(centroids):
    """[M, D] -> fp8 [D, M] with dim D-1 carrying delta = 512 - csq/2."""
    centroids = np.asarray(centroids)
    csq = np.einsum("md,md->m", centroids.astype(np.float64),
                    centroids.astype(np.float64))
    cd = centroids.copy()
    cd[:, D - 1] = (BIAS_CENTER - csq * 0.5).astype(np.float32)
    return np.ascontiguousarray(cd.astype(NP_FP8).T)                # [D, M]


def prep_rows(e):
    """[R, D] fp32 rows -> (et fp8 [D, R], feat fp32 [128, NT])."""
    f = np.einsum("rd,rd->r", e.astype(np.float64),
                  e.astype(np.float64)) + 2.0 * BIAS_CENTER
    ed = e.copy()
    ed[:, D - 1] = 1.0
    et_np = np.ascontiguousarray(ed.astype(NP_FP8).T)               # [D, R]
    feat_np = np.ascontiguousarray(
        f.astype(np.float32).reshape(-1, 128).T)                    # [128, NT]
    return et_np, feat_np


def prep_inputs(embeds, centroids):
    """Host-side shard + layout prep. Returns per-core input maps."""
    embeds = np.asarray(embeds)
    ct_np = prep_centroids(centroids)
    in_maps = []
    for c in range(N_CORES):
        e = embeds[c * B_PER_CORE:(c + 1) * B_PER_CORE].reshape(R, D)
        et_np, feat_np = prep_rows(e)
        in_maps.append({"et": et_np, "ct": ct_np, "feat": feat_np})
    return in_maps


def gather_output(results):
    """results: list of 8 dicts with 'out' [128, NT] -> [B, 1, 56, 56]."""
    per_core = [np.asarray(r["out"]).T.reshape(R) for r in results]
    sim = np.concatenate(per_core).reshape(B, N)
    return sim.reshape(B, FP_H, FP_H)[:, None, :, :].astype(np.float32)


def kernel(embeds, centroids):
    nc = _get_program()
    in_maps = prep_inputs(embeds, centroids)
    res = run_bass_kernel_spmd(nc, in_maps, list(range(N_CORES)))
    return gather_output(res.results)


class CachedRunner:
    """Low-overhead repeat runner: jit once, keep inputs resident on device.

    Mirrors bass2jax.run_bass_via_pjrt's multi-core path but caches the
    jitted callable and the device-side input shards so repeated calls pay
    only dispatch + execution (for timing measurements).
    """

    def __init__(self, nc, in_maps):
        import jax
        import concourse.mybir as _mybir
        from jax.sharding import Mesh, PartitionSpec, NamedSharding
        from jax.experimental.shard_map import shard_map
        from concourse import bass2jax

        bass2jax.install_neuronx_cc_hook()
        n_cores = len(in_maps)
        partition_name = (nc.partition_id_tensor.name
                          if nc.partition_id_tensor else None)
        in_names, out_names, out_avals = [], [], []
        for alloc in nc.m.functions[0].allocations:
            if not isinstance(alloc, _mybir.MemoryLocationSet):
                continue
            name = alloc.memorylocations[0].name
            if alloc.kind == "ExternalInput":
                if name != partition_name:
                    in_names.append(name)
            elif alloc.kind == "ExternalOutput":
                shape = tuple(alloc.tensor_shape)
                dtype = _mybir.dt.np(alloc.dtype)
                out_names.append(name)
                out_avals.append(jax.core.ShapedArray(shape, dtype))
        n_params = len(in_names)
        all_in = in_names + out_names
        if partition_name is not None:
            all_in.append(partition_name)

        def _body(*args):
            operands = list(args)
            if partition_name is not None:
                operands.append(bass2jax.partition_id_tensor())
            outs = bass2jax._bass_exec_p.bind(
                *operands,
                out_avals=tuple(out_avals),
                in_names=tuple(all_in),
                out_names=tuple(out_names),
                lowering_input_output_aliases=(),
                sim_require_finite=True,
                sim_require_nnan=True,
                nc=nc,
            )
            return tuple(outs)

        devices = jax.devices()[:n_cores]
        mesh = Mesh(np.asarray(devices), ("core",))
        n_outs = len(out_names)
        donate = tuple(range(n_params, n_params + n_outs))
        self._fn = jax.jit(
            shard_map(_body, mesh=mesh,
                      in_specs=(PartitionSpec("core"),) * (n_params + n_outs),
                      out_specs=(PartitionSpec("core"),) * n_outs,
                      check_rep=False),
            donate_argnums=donate, keep_unused=True)
        sh = NamedSharding(mesh, PartitionSpec("core"))
        self._dev_in = [
            jax.device_put(
                np.concatenate([np.asarray(in_maps[c][nm])
                                for c in range(n_cores)], axis=0), sh)
            for nm in in_names]
        self._zero_shapes = [(n_cores * a.shape[0], *a.shape[1:])
                             for a in out_avals]
        self._zero_dtypes = [a.dtype for a in out_avals]
        self._out_names = out_names
        self._out_avals = out_avals
        self._n_cores = n_cores
        self._jax = jax

    def __call__(self):
        zeros = [np.zeros(s, d) for s, d in
                 zip(self._zero_shapes, self._zero_dtypes)]
        out = self._fn(*self._dev_in, *zeros)
        self._jax.block_until_ready(out)
        return out

    def results(self):
        out = self()
        return [
            {nm: np.asarray(out[i]).reshape(
                self._n_cores, *self._out_avals[i].shape)[c]
             for i, nm in enumerate(self._out_names)}
            for c in range(self._n_cores)]
